# revision 1
# baseline (speedup 1.0000x reference)
"""Trainium2 Bass kernel for nn_Conv2d_22222160789797.

Conv2d: x [32,128,56,56] f32, weight [256,128,3,3] (OIHW), stride 1, pad 1
-> out [32,256,56,56] f32.

Strategy: data-parallel over batch across 8 cores (4 images/core). Per core,
the conv is 9 accumulating matmuls per output tile: contract over in-channels
(partition dim K=128) with the weight slice for each (kh,kw) tap as the
stationary operand and a shifted window of the zero-padded input as the moving
operand. fp32r matmuls run at 1 cycle/row for N>=256 (4x faster than fp32)
with ~1.5e-4 rms relative error.

Host prep: zero-pad x to 58x58 (so no on-device memset/edge handling) and
transpose weight to [ic, (kh kw) oc] so the lhsT slices are contiguous SBUF
columns. Output: per image and out-channel half, 7 chunks of 8 output rows
(N = 8*56 = 448 <= 512 PSUM bank limit), PSUM -> SBUF copy on DVE, then
contiguous DMA to HBM.
"""

import numpy as np

import concourse.tile as tile
from concourse import bacc, mybir
from concourse.bass_utils import run_bass_kernel_spmd

N_CORES = 8
B, IC, H, W = 32, 128, 56, 56
OC, KH, KW = 256, 3, 3
BPC = B // N_CORES          # images per core
PH, PW = H + 2, W + 2       # padded 58x58
ROWS_PER_CHUNK = 8
N_CHUNKS = H // ROWS_PER_CHUNK  # 7
OC_HALVES = OC // 128       # 2

_f32 = mybir.dt.float32
_f32r = mybir.dt.float32r

_compiled_nc = None


BAND_ROWS = ROWS_PER_CHUNK + 2  # 10 padded rows cover one chunk's taps
N_WARMUP = 10  # dummy matmuls to lift the PE HAM clock gate during the head


def _build(reps=1, warmup=N_WARMUP):
    """reps>1 repeats the whole conv body (same inputs/outputs) inside one
    NEFF — used only for benchmarking kernel time without NTFF profiling.

    DMA plan: each `dma_start` costs ~650ns of sequencer issue time, so input
    loads are few and big, on the sync (SP HWDGE) ring, ordered so the first
    accumulation group's deps land first: weight half 0 (one strided DMA),
    then image-0 row bands, weight half 1, then whole-image DMAs for images
    1-3 (prefetched under compute). The 56 per-chunk output DMAs go on the
    scalar (ACT HWDGE) ring, which is otherwise idle. Dummy matmuls on a
    zeroed scratch tile bridge the initial DMA wait so the PE's HAM clock
    gate is already at full rate when the real matmuls start."""
    nc = bacc.Bacc("TRN2", target_bir_lowering=False, debug=False)
    x_d = nc.dram_tensor("x", [BPC, IC, PH, PW], _f32r, kind="ExternalInput")
    w_d = nc.dram_tensor("w", [IC, KH * KW * OC], _f32r, kind="ExternalInput")
    o_d = nc.dram_tensor("out", [BPC, OC, H, W], _f32, kind="ExternalOutput")
    # view for strided per-half weight loads: [ic, tap, oc]
    w3 = w_d[:].rearrange("p (k c) -> p k c", k=KH * KW, c=OC)

    with tile.TileContext(nc) as tc:
        with (
            tc.tile_pool(name="w", bufs=1) as wpool,
            tc.tile_pool(name="x", bufs=1) as xpool,
            tc.tile_pool(name="o", bufs=4) as opool,
            tc.tile_pool(name="ps", bufs=8, space="PSUM") as pspool,
        ):
            if warmup:
                _bf16 = mybir.dt.bfloat16
                wscr = wpool.tile([128, 128], _bf16, name="wscr", tag="wscr")
                xscr = wpool.tile([128, ROWS_PER_CHUNK * W], _bf16,
                                  name="xscr", tag="xscr")
                nc.gpsimd.memset(wscr[:], 0.0)
                nc.gpsimd.memset(xscr[:], 0.0)
                pwarm = pspool.tile([128, ROWS_PER_CHUNK * W], _f32,
                                    name="pwarm", tag="ps")
                for _ in range(warmup):
                    nc.tensor.matmul(pwarm[:], wscr[:], xscr[:],
                                     start=True, stop=True)

            # Weight halves on the sync ring; image-0 bands + output DMAs on
            # the scalar ring — the two first-group deps (wh0, band0)
            # transfer in parallel on separate HWDGE rings. A group only
            # starts once its whole weight half is resident (partial-tap
            # delivery stalls mid-accumulation, measured slower).
            wh = []
            for half in range(OC_HALVES):
                t = wpool.tile([IC, KH * KW, 128], _f32r, name=f"wh{half}",
                               tag=f"wh{half}")
                wh.append(t)
            nc.sync.dma_start(wh[0][:], w3[:, :, 0:128])

            def tap(half, k):
                return wh[half][:, k, :]

            # image 0 as 7 overlapping row-band tiles (each chunk's matmuls
            # gate on one ~300KB band instead of the whole 1.7MB image)
            bands0 = []
            for ch in range(N_CHUNKS):
                b = xpool.tile([IC, BAND_ROWS, PW], _f32r, name="band",
                               tag="band", bufs=N_CHUNKS)
                nc.sync.dma_start(
                    b[:],
                    x_d[0, :, ch * ROWS_PER_CHUNK : ch * ROWS_PER_CHUNK
                        + BAND_ROWS, :],
                )
                bands0.append(b)
            nc.sync.dma_start(wh[1][:], w3[:, :, 128:256])

            def chunk_group(rhs_tile, row_off, img, half, ch):
                ps = pspool.tile([128, ROWS_PER_CHUNK, W], _f32,
                                 name="ps", tag="ps")
                for k in range(KH * KW):
                    kh, kw = divmod(k, KW)
                    r = row_off + kh
                    nc.tensor.matmul(
                        ps[:],
                        tap(half, k),
                        rhs_tile[:, r : r + ROWS_PER_CHUNK, kw : kw + W],
                        start=(k == 0),
                        stop=(k == KH * KW - 1),
                    )
                r0 = ch * ROWS_PER_CHUNK
                ot = opool.tile([128, ROWS_PER_CHUNK, W], _f32,
                                name="ot", tag="ot")
                nc.vector.tensor_copy(ot[:], ps[:])
                nc.scalar.dma_start(
                    o_d[img, half * 128 : half * 128 + 128,
                        r0 : r0 + ROWS_PER_CHUNK, :],
                    ot[:],
                )

            for _rep in range(reps):
                for img in range(BPC):
                    if img == 0 and _rep == 0:
                        for half in range(OC_HALVES):
                            for ch in range(N_CHUNKS):
                                chunk_group(bands0[ch], 0, img, half, ch)
                    else:
                        xt = xpool.tile([IC, PH, PW], _f32r, name="xt",
                                        tag="xt", bufs=2)
                        nc.sync.dma_start(xt[:], x_d[img])
                        for half in range(OC_HALVES):
                            for ch in range(N_CHUNKS):
                                chunk_group(xt, ch * ROWS_PER_CHUNK,
                                            img, half, ch)
    nc.compile()
    return nc


def _get_nc():
    global _compiled_nc
    if _compiled_nc is None:
        _compiled_nc = _build()
    return _compiled_nc


def _prep_inputs(x, weight):
    x = np.asarray(x, dtype=np.float32)
    weight = np.asarray(weight, dtype=np.float32)
    xp = np.zeros((B, IC, PH, PW), dtype=np.float32)
    xp[:, :, 1 : H + 1, 1 : W + 1] = x
    # [oc, ic, kh, kw] -> [ic, kh, kw, oc] -> [ic, (kh kw oc)]
    wt = np.ascontiguousarray(weight.transpose(1, 2, 3, 0)).reshape(IC, KH * KW * OC)
    in_maps = [
        {"x": np.ascontiguousarray(xp[c * BPC : (c + 1) * BPC]), "w": wt}
        for c in range(N_CORES)
    ]
    return in_maps


def _run(x, weight, trace=False):
    nc = _get_nc()
    in_maps = _prep_inputs(x, weight)
    res = run_bass_kernel_spmd(nc, in_maps, list(range(N_CORES)), trace=trace)
    out = np.concatenate([res.results[c]["out"] for c in range(N_CORES)], axis=0)
    return out, res


def kernel(x, weight):
    out, _ = _run(x, weight)
    return out



# revision 9
# speedup vs baseline: 1.0766x; 1.0766x over previous
"""Trainium2 Bass kernel for nn_Conv2d_22222160789797.

Conv2d: x [32,128,56,56] f32, weight [256,128,3,3] (OIHW), stride 1, pad 1
-> out [32,256,56,56] f32.

Strategy: data-parallel over batch across 8 cores (4 images/core). Per core,
the conv is 9 accumulating matmuls per output tile: contract over in-channels
(partition dim K=128) with the weight slice for each (kh,kw) tap as the
stationary operand and a shifted window of the zero-padded input as the moving
operand. Inputs are cast to bf16 on the host: bf16 matmuls stream 1 row/cycle
like fp32r, but LDWEIGHTS gets fast-weight-load (fp32r's 4-byte weight load
measured 224ns/matmul and capped the PE at 253ns/MM vs the 187ns stream
ideal), and input DMA bytes halve. ~1e-3 max relative error.

Host prep: zero-pad x to 58x58 (so no on-device memset/edge handling) and
transpose weight to [ic, (kh kw) oc] so the lhsT slices are contiguous SBUF
columns. Output: per image and out-channel half, 7 chunks of 8 output rows
(N = 8*56 = 448 <= 512 PSUM bank limit), PSUM -> SBUF copy on DVE, then
contiguous DMA to HBM.
"""

import ml_dtypes
import numpy as np

import concourse.tile as tile
from concourse import bacc, mybir
from concourse.bass_utils import run_bass_kernel_spmd

N_CORES = 8
B, IC, H, W = 32, 128, 56, 56
OC, KH, KW = 256, 3, 3
BPC = B // N_CORES          # images per core
PH, PW = H + 2, W + 2       # padded 58x58
ROWS_PER_CHUNK = 8
N_CHUNKS = H // ROWS_PER_CHUNK  # 7
OC_HALVES = OC // 128       # 2

_f32 = mybir.dt.float32
_bf16 = mybir.dt.bfloat16
_bf16_np = ml_dtypes.bfloat16

_compiled_nc = None


BAND_ROWS = ROWS_PER_CHUNK + 2  # 10 padded rows cover one chunk's taps
N_WARMUP = 4  # dummy matmuls to lift the PE HAM clock gate during the head


def _build(reps=1, warmup=N_WARMUP):
    """reps>1 repeats the whole conv body (same inputs/outputs) inside one
    NEFF — used only for benchmarking kernel time without NTFF profiling.

    DMA plan: each `dma_start` costs ~650ns of sequencer issue time, so input
    loads are few and big, on the sync (SP HWDGE) ring, ordered so the first
    accumulation group's deps land first: weight half 0 (one strided DMA),
    then image-0 row bands, weight half 1, then whole-image DMAs for images
    1-3 (prefetched under compute). The 56 per-chunk output DMAs go on the
    scalar (ACT HWDGE) ring, which is otherwise idle. Dummy matmuls on a
    zeroed scratch tile bridge the initial DMA wait so the PE's HAM clock
    gate is already at full rate when the real matmuls start."""
    nc = bacc.Bacc("TRN2", target_bir_lowering=False, debug=False)
    x_d = nc.dram_tensor("x", [BPC, IC, PH, PW], _bf16, kind="ExternalInput")
    w_d = nc.dram_tensor("w", [IC, KH * KW * OC], _bf16, kind="ExternalInput")
    o_d = nc.dram_tensor("out", [BPC, OC, H, W], _f32, kind="ExternalOutput")
    # view for strided per-half weight loads: [ic, tap, oc]
    w3 = w_d[:].rearrange("p (k c) -> p k c", k=KH * KW, c=OC)

    with tile.TileContext(nc) as tc:
        with (
            tc.tile_pool(name="w", bufs=1) as wpool,
            tc.tile_pool(name="x", bufs=1) as xpool,
            tc.tile_pool(name="o", bufs=4) as opool,
            tc.tile_pool(name="ps", bufs=8, space="PSUM") as pspool,
        ):
            if warmup:
                wscr = wpool.tile([128, 128], _bf16, name="wscr", tag="wscr")
                xscr = wpool.tile([128, ROWS_PER_CHUNK * W], _bf16,
                                  name="xscr", tag="xscr")
                nc.gpsimd.memset(wscr[:], 0.0)
                nc.gpsimd.memset(xscr[:], 0.0)
                pwarm = pspool.tile([128, ROWS_PER_CHUNK * W], _f32,
                                    name="pwarm", tag="ps")
                for _ in range(warmup):
                    nc.tensor.matmul(pwarm[:], wscr[:], xscr[:],
                                     start=True, stop=True)

            # Weight halves on the sync ring; image-0 bands + output DMAs on
            # the scalar ring — the two first-group deps (wh0, band0)
            # transfer in parallel on separate HWDGE rings. A group only
            # starts once its whole weight half is resident (partial-tap
            # delivery stalls mid-accumulation, measured slower).
            wh = []
            for half in range(OC_HALVES):
                t = wpool.tile([IC, KH * KW, 128], _bf16, name=f"wh{half}",
                               tag=f"wh{half}")
                wh.append(t)
            nc.sync.dma_start(wh[0][:], w3[:, :, 0:128])

            def tap(half, k):
                return wh[half][:, k, :]

            # image 0 as 7 overlapping row-band tiles (each chunk's matmuls
            # gate on one ~300KB band instead of the whole 1.7MB image)
            bands0 = []
            for ch in range(N_CHUNKS):
                b = xpool.tile([IC, BAND_ROWS, PW], _bf16, name="band",
                               tag="band", bufs=N_CHUNKS)
                nc.sync.dma_start(
                    b[:],
                    x_d[0, :, ch * ROWS_PER_CHUNK : ch * ROWS_PER_CHUNK
                        + BAND_ROWS, :],
                )
                bands0.append(b)
            nc.sync.dma_start(wh[1][:], w3[:, :, 128:256])

            def chunk_group(rhs_tile, row_off, img, half, ch):
                ps = pspool.tile([128, ROWS_PER_CHUNK, W], _f32,
                                 name="ps", tag="ps")
                for k in range(KH * KW):
                    kh, kw = divmod(k, KW)
                    r = row_off + kh
                    nc.tensor.matmul(
                        ps[:],
                        tap(half, k),
                        rhs_tile[:, r : r + ROWS_PER_CHUNK, kw : kw + W],
                        start=(k == 0),
                        stop=(k == KH * KW - 1),
                    )
                r0 = ch * ROWS_PER_CHUNK
                ot = opool.tile([128, ROWS_PER_CHUNK, W], _f32,
                                name="ot", tag="ot")
                nc.vector.tensor_copy(ot[:], ps[:])
                nc.scalar.dma_start(
                    o_d[img, half * 128 : half * 128 + 128,
                        r0 : r0 + ROWS_PER_CHUNK, :],
                    ot[:],
                )

            for _rep in range(reps):
                for img in range(BPC):
                    if img == 0 and _rep == 0:
                        for half in range(OC_HALVES):
                            for ch in range(N_CHUNKS):
                                chunk_group(bands0[ch], 0, img, half, ch)
                    else:
                        xt = xpool.tile([IC, PH, PW], _bf16, name="xt",
                                        tag="xt", bufs=2)
                        nc.sync.dma_start(xt[:], x_d[img])
                        for half in range(OC_HALVES):
                            for ch in range(N_CHUNKS):
                                chunk_group(xt, ch * ROWS_PER_CHUNK,
                                            img, half, ch)
    nc.compile()
    return nc


def _get_nc():
    global _compiled_nc
    if _compiled_nc is None:
        _compiled_nc = _build()
    return _compiled_nc


def _prep_inputs(x, weight):
    x = np.asarray(x, dtype=np.float32)
    weight = np.asarray(weight, dtype=np.float32)
    xp = np.zeros((B, IC, PH, PW), dtype=_bf16_np)
    xp[:, :, 1 : H + 1, 1 : W + 1] = x
    # [oc, ic, kh, kw] -> [ic, kh, kw, oc] -> [ic, (kh kw oc)]
    wt = np.ascontiguousarray(
        weight.transpose(1, 2, 3, 0).astype(_bf16_np)
    ).reshape(IC, KH * KW * OC)
    in_maps = [
        {"x": np.ascontiguousarray(xp[c * BPC : (c + 1) * BPC]), "w": wt}
        for c in range(N_CORES)
    ]
    return in_maps


def _run(x, weight, trace=False):
    nc = _get_nc()
    in_maps = _prep_inputs(x, weight)
    res = run_bass_kernel_spmd(nc, in_maps, list(range(N_CORES)), trace=trace)
    out = np.concatenate([res.results[c]["out"] for c in range(N_CORES)], axis=0)
    return out, res


def kernel(x, weight):
    out, _ = _run(x, weight)
    return out



# revision 11
# speedup vs baseline: 1.1982x; 1.1129x over previous
"""Trainium2 Bass kernel for nn_Conv2d_22222160789797.

Conv2d: x [32,128,56,56] f32, weight [256,128,3,3] (OIHW), stride 1, pad 1
-> out [32,256,56,56] f32.

Strategy: data-parallel over batch across 8 cores (4 images/core). Per core,
the conv is 9 accumulating matmuls per output tile: contract over in-channels
(partition dim K=128) with the weight slice for each (kh,kw) tap as the
stationary operand and a shifted window of the zero-padded input as the moving
operand. Inputs are cast to bf16 on the host (~2e-3 max rel err, limit 2e-2).

Loop order is tap-outer / chunk-inner: each (image, oc-half) group keeps 7
PSUM banks live (one per 8-row output chunk, N=448 fp32 <= 512 bank limit)
and sweeps the 9 taps over all 7 chunks, so the stationary weight is reused
across 7 consecutive matmuls. With per-matmul weight swaps the PE issue rate
measured 237ns (fp32r: 253ns); the LDWEIGHTS handoff, not the 448-cycle
stream (187ns), was the limiter.

DMA plan: weight as one contiguous [128, 4608B] DMA on the scalar ring;
whole images (6728B/partition contiguous) on the sync ring, double-buffered.
PSUM evacuation alternates DVE/ACT per chunk so bank-free keeps pace with
the next group's matmuls; output DMAs ride the scalar ring. Dummy matmuls on
a zeroed scratch tile bridge the initial DMA wait so the PE HAM clock gate
is at full rate when the real matmuls start.
"""

import ml_dtypes
import numpy as np

import concourse.tile as tile
from concourse import bacc, mybir
from concourse.bass_utils import run_bass_kernel_spmd

N_CORES = 8
B, IC, H, W = 32, 128, 56, 56
OC, KH, KW = 256, 3, 3
BPC = B // N_CORES          # images per core
PH, PW = H + 2, W + 2       # padded 58x58
ROWS_PER_CHUNK = 8
N_CHUNKS = H // ROWS_PER_CHUNK  # 7
OC_HALVES = OC // 128       # 2
NTAPS = KH * KW

_f32 = mybir.dt.float32
_bf16 = mybir.dt.bfloat16
_bf16_np = ml_dtypes.bfloat16

_compiled_nc = None

N_WARMUP = 6  # dummy matmuls covering the initial DMA wait (~2.6us cold)


def _build(warmup=N_WARMUP):
    nc = bacc.Bacc("TRN2", target_bir_lowering=False, debug=False)
    x_d = nc.dram_tensor("x", [BPC, IC, PH, PW], _bf16, kind="ExternalInput")
    w_d = nc.dram_tensor("w", [IC, NTAPS * OC], _bf16, kind="ExternalInput")
    o_d = nc.dram_tensor("out", [BPC, OC, H, W], _f32, kind="ExternalOutput")
    w3 = w_d[:].rearrange("p (k c) -> p k c", k=NTAPS, c=OC)

    with tile.TileContext(nc) as tc:
        with (
            tc.tile_pool(name="w", bufs=1) as wpool,
            tc.tile_pool(name="x", bufs=1) as xpool,
            tc.tile_pool(name="o", bufs=1) as opool,
            tc.tile_pool(name="ps", bufs=8, space="PSUM") as pspool,
        ):
            if warmup:
                wscr = wpool.tile([128, 128], _bf16, name="wscr", tag="wscr")
                xscr = wpool.tile([128, ROWS_PER_CHUNK * W], _bf16,
                                  name="xscr", tag="xscr")
                nc.gpsimd.memset(wscr[:], 0.0)
                nc.gpsimd.memset(xscr[:], 0.0)
                pwarm = pspool.tile([128, ROWS_PER_CHUNK * W], _f32,
                                    name="pwarm", tag="ps")
                for _ in range(warmup):
                    nc.tensor.matmul(pwarm[:], wscr[:], xscr[:],
                                     start=True, stop=True)

            # whole weight, one contiguous DMA (4608B per partition) on the
            # scalar ring; images on the sync ring so both transfer at once
            wt = wpool.tile([IC, NTAPS, OC], _bf16, name="wt", tag="wt")
            nc.scalar.dma_start(wt[:], w3)

            def tap(half, k):
                return wt[:, k, half * 128 : half * 128 + 128]

            for img in range(BPC):
                xt = xpool.tile([IC, PH, PW], _bf16, name="xt",
                                tag="xt", bufs=2)
                nc.sync.dma_start(xt[:], x_d[img])
                for half in range(OC_HALVES):
                    pss = []
                    for ch in range(N_CHUNKS):
                        ps = pspool.tile([128, ROWS_PER_CHUNK, W], _f32,
                                         name="ps", tag="ps")
                        pss.append(ps)
                    for k in range(NTAPS):
                        kh, kw = divmod(k, KW)
                        for ch in range(N_CHUNKS):
                            r = ch * ROWS_PER_CHUNK + kh
                            nc.tensor.matmul(
                                pss[ch][:],
                                tap(half, k),
                                xt[:, r : r + ROWS_PER_CHUNK, kw : kw + W],
                                start=(k == 0),
                                stop=(k == NTAPS - 1),
                            )
                    for ch in range(N_CHUNKS):
                        r0 = ch * ROWS_PER_CHUNK
                        if ch % 2 == 0:
                            ot = opool.tile([128, ROWS_PER_CHUNK, W], _f32,
                                            name="otv", tag="otv", bufs=3)
                            nc.vector.tensor_copy(ot[:], pss[ch][:])
                        else:
                            ot = opool.tile([128, ROWS_PER_CHUNK, W], _f32,
                                            name="ots", tag="ots", bufs=3)
                            nc.scalar.copy(ot[:], pss[ch][:])
                        nc.scalar.dma_start(
                            o_d[img, half * 128 : half * 128 + 128,
                                r0 : r0 + ROWS_PER_CHUNK, :],
                            ot[:],
                        )
    nc.compile()
    return nc


def _get_nc():
    global _compiled_nc
    if _compiled_nc is None:
        _compiled_nc = _build()
    return _compiled_nc


def _prep_inputs(x, weight):
    x = np.asarray(x, dtype=np.float32)
    weight = np.asarray(weight, dtype=np.float32)
    xp = np.zeros((B, IC, PH, PW), dtype=_bf16_np)
    xp[:, :, 1 : H + 1, 1 : W + 1] = x
    # [oc, ic, kh, kw] -> [ic, kh, kw, oc] -> [ic, (kh kw oc)]
    wt = np.ascontiguousarray(
        weight.transpose(1, 2, 3, 0).astype(_bf16_np)
    ).reshape(IC, NTAPS * OC)
    in_maps = [
        {"x": np.ascontiguousarray(xp[c * BPC : (c + 1) * BPC]), "w": wt}
        for c in range(N_CORES)
    ]
    return in_maps


def _run(x, weight, trace=False):
    nc = _get_nc()
    in_maps = _prep_inputs(x, weight)
    res = run_bass_kernel_spmd(nc, in_maps, list(range(N_CORES)), trace=trace)
    out = np.concatenate([res.results[c]["out"] for c in range(N_CORES)], axis=0)
    return out, res


def kernel(x, weight):
    out, _ = _run(x, weight)
    return out


# revision 14
# speedup vs baseline: 1.2307x; 1.0271x over previous
"""Trainium2 Bass kernel for nn_Conv2d_22222160789797.

Conv2d: x [32,128,56,56] f32, weight [256,128,3,3] (OIHW), stride 1, pad 1
-> out [32,256,56,56] f32.

Strategy: data-parallel over batch across 8 cores (4 images/core). Per core,
the conv is 9 accumulating matmuls per output tile: contract over in-channels
(partition dim K=128) with the weight slice for each (kh,kw) tap as the
stationary operand and a shifted window of the zero-padded input as the moving
operand. Inputs are cast to bf16 on the host (~2e-3 max rel err, limit 2e-2).

Loop order is tap-outer / chunk-inner: each (image, oc-half) group keeps 7
PSUM banks live (one per 8-row output chunk, N=448 fp32 <= 512 bank limit)
and sweeps the 9 taps over all 7 chunks, so the stationary weight is reused
across 7 consecutive matmuls and LDWEIGHTS hides under the 448-cycle stream.
Measured steady-state matmul issue gap: 192ns (187ns streaming floor); with
per-matmul weight swaps it was 237ns, with fp32r 4-byte weights 253ns.

Head/tail plumbing (all measured bottlenecks in earlier traces):
- weight halves are host-relaid contiguous ([ic, half, tap, 128]) and ride
  the otherwise-idle vector DGE ring, so the first half lands ~1us after
  user code starts instead of queuing behind image bytes.
- image 0 is split into two contiguous row-slabs (rows 0-33 / 32-57) so the
  first group's matmuls gate on a 0.46MB transfer, not the whole 0.86MB
  image; images 1-3 are whole-image DMAs double-buffered on the sync ring.
- PSUM evacuation alternates DVE/ACT per chunk so bank-free keeps pace with
  the next group's matmuls; output DMAs ride the sync ring (the ACT ring
  backed up behind its copies and cost an 8.7us tail when it carried them).
- dummy matmuls on a zeroed scratch tile bridge the initial DMA wait so the
  PE HAM clock gate is at full rate when the real matmuls start.
"""

import ml_dtypes
import numpy as np

import concourse.tile as tile
from concourse import bacc, mybir
from concourse.bass_utils import run_bass_kernel_spmd

N_CORES = 8
B, IC, H, W = 32, 128, 56, 56
OC, KH, KW = 256, 3, 3
BPC = B // N_CORES          # images per core
PH, PW = H + 2, W + 2       # padded 58x58
ROWS_PER_CHUNK = 8
N_CHUNKS = H // ROWS_PER_CHUNK  # 7
OC_HALVES = OC // 128       # 2
NTAPS = KH * KW
SLAB_SPLIT = 32             # img0 slab A = rows 0..33, slab B = rows 32..57
SLAB_A_ROWS = SLAB_SPLIT + 2
SLAB_B_ROWS = PH - SLAB_SPLIT

_f32 = mybir.dt.float32
_bf16 = mybir.dt.bfloat16
_bf16_np = ml_dtypes.bfloat16

_compiled_nc = None

N_WARMUP = 7  # dummy matmuls covering the initial DMA wait (~2.6us cold)


def _build(warmup=N_WARMUP):
    nc = bacc.Bacc("TRN2", target_bir_lowering=False, debug=False)
    x_d = nc.dram_tensor("x", [BPC, IC, PH, PW], _bf16, kind="ExternalInput")
    w_d = nc.dram_tensor("w", [IC, OC_HALVES * NTAPS * 128], _bf16,
                         kind="ExternalInput")
    o_d = nc.dram_tensor("out", [BPC, OC, H, W], _f32, kind="ExternalOutput")
    w4 = w_d[:].rearrange("p (h k c) -> p h k c", h=OC_HALVES, k=NTAPS, c=128)

    with tile.TileContext(nc) as tc:
        with (
            tc.tile_pool(name="w", bufs=1) as wpool,
            tc.tile_pool(name="x", bufs=1) as xpool,
            tc.tile_pool(name="o", bufs=1) as opool,
            tc.tile_pool(name="ps", bufs=8, space="PSUM") as pspool,
        ):
            if warmup:
                wscr = wpool.tile([128, 128], _bf16, name="wscr", tag="wscr")
                xscr = wpool.tile([128, ROWS_PER_CHUNK * W], _bf16,
                                  name="xscr", tag="xscr")
                nc.gpsimd.memset(wscr[:], 0.0)
                nc.gpsimd.memset(xscr[:], 0.0)
                pwarm = pspool.tile([128, ROWS_PER_CHUNK * W], _f32,
                                    name="pwarm", tag="ps")
                for _ in range(warmup):
                    nc.tensor.matmul(pwarm[:], wscr[:], xscr[:],
                                     start=True, stop=True)

            # weight halves, each contiguous (2304B/partition), via the idle
            # gpsimd SWDGE so they don't queue behind image bytes; half 0 is
            # the first-needed dependency
            wh = []
            for half in range(OC_HALVES):
                t = wpool.tile([IC, NTAPS, 128], _bf16, name=f"wh{half}",
                               tag=f"wh{half}")
                nc.gpsimd.dma_start(t[:], w4[:, half])
                wh.append(t)

            def tap(half, k):
                return wh[half][:, k, :]

            # image 0 as two contiguous row-slabs so the first matmuls gate
            # on less than half the image's bytes
            x0a = xpool.tile([IC, SLAB_A_ROWS, PW], _bf16, name="x0a",
                             tag="x0a")
            nc.sync.dma_start(x0a[:], x_d[0, :, 0:SLAB_A_ROWS, :])
            x0b = xpool.tile([IC, SLAB_B_ROWS, PW], _bf16, name="x0b",
                             tag="x0b")
            nc.sync.dma_start(x0b[:], x_d[0, :, SLAB_SPLIT:PH, :])

            def img0_rhs(ch, kh, kw):
                r = ch * ROWS_PER_CHUNK + kh
                if ch * ROWS_PER_CHUNK < SLAB_SPLIT:
                    return x0a[:, r : r + ROWS_PER_CHUNK, kw : kw + W]
                r -= SLAB_SPLIT
                return x0b[:, r : r + ROWS_PER_CHUNK, kw : kw + W]

            # images 1-3 fully prefetched up front (own buffers, no reuse
            # waits) so no image DMA ever queues behind output DMAs on the
            # sync ring
            rhs_fns = [img0_rhs]
            for img in range(1, BPC):
                xt = xpool.tile([IC, PH, PW], _bf16, name=f"xt{img}",
                                tag=f"xt{img}")
                nc.sync.dma_start(xt[:], x_d[img])

                def rhs_of(ch, kh, kw, _xt=xt):
                    r = ch * ROWS_PER_CHUNK + kh
                    return _xt[:, r : r + ROWS_PER_CHUNK, kw : kw + W]

                rhs_fns.append(rhs_of)

            for img in range(BPC):
                rhs_of = rhs_fns[img]
                for half in range(OC_HALVES):
                    pss = []
                    for ch in range(N_CHUNKS):
                        ps = pspool.tile([128, ROWS_PER_CHUNK, W], _f32,
                                         name="ps", tag="ps")
                        pss.append(ps)
                    for k in range(NTAPS):
                        kh, kw = divmod(k, KW)
                        for ch in range(N_CHUNKS):
                            nc.tensor.matmul(
                                pss[ch][:],
                                tap(half, k),
                                rhs_of(ch, kh, kw),
                                start=(k == 0),
                                stop=(k == NTAPS - 1),
                            )
                    for ch in range(N_CHUNKS):
                        r0 = ch * ROWS_PER_CHUNK
                        if ch % 2 == 0:
                            ot = opool.tile([128, ROWS_PER_CHUNK, W], _f32,
                                            name="otv", tag="otv", bufs=3)
                            nc.vector.tensor_copy(ot[:], pss[ch][:])
                        else:
                            ot = opool.tile([128, ROWS_PER_CHUNK, W], _f32,
                                            name="ots", tag="ots", bufs=3)
                            nc.scalar.copy(ot[:], pss[ch][:])
                        nc.sync.dma_start(
                            o_d[img, half * 128 : half * 128 + 128,
                                r0 : r0 + ROWS_PER_CHUNK, :],
                            ot[:],
                        )
    nc.compile()
    return nc


def _get_nc():
    global _compiled_nc
    if _compiled_nc is None:
        _compiled_nc = _build()
    return _compiled_nc


def _prep_inputs(x, weight):
    x = np.asarray(x, dtype=np.float32)
    weight = np.asarray(weight, dtype=np.float32)
    xp = np.zeros((B, IC, PH, PW), dtype=_bf16_np)
    xp[:, :, 1 : H + 1, 1 : W + 1] = x
    # [oc, ic, kh, kw] -> [ic, oc-half, kh*kw, 128] -> [ic, flat]
    wt = weight.transpose(1, 0, 2, 3).reshape(IC, OC_HALVES, 128, NTAPS)
    wt = np.ascontiguousarray(wt.transpose(0, 1, 3, 2).astype(_bf16_np))
    wt = wt.reshape(IC, OC_HALVES * NTAPS * 128)
    in_maps = [
        {"x": np.ascontiguousarray(xp[c * BPC : (c + 1) * BPC]), "w": wt}
        for c in range(N_CORES)
    ]
    return in_maps


def _run(x, weight, trace=False):
    nc = _get_nc()
    in_maps = _prep_inputs(x, weight)
    res = run_bass_kernel_spmd(nc, in_maps, list(range(N_CORES)), trace=trace)
    out = np.concatenate([res.results[c]["out"] for c in range(N_CORES)], axis=0)
    return out, res


def kernel(x, weight):
    out, _ = _run(x, weight)
    return out


# revision 17
# speedup vs baseline: 1.2546x; 1.0195x over previous
"""Trainium2 Bass kernel for nn_Conv2d_22222160789797.

Conv2d: x [32,128,56,56] f32, weight [256,128,3,3] (OIHW), stride 1, pad 1
-> out [32,256,56,56] f32.

Strategy: data-parallel over batch across 8 cores (4 images/core). Per core,
the conv is 9 accumulating matmuls per output tile: contract over in-channels
(partition dim K=128) with the weight slice for each (kh,kw) tap as the
stationary operand and a shifted window of the zero-padded input as the moving
operand. Inputs are cast to bf16 on the host (~2e-3 max rel err, limit 2e-2).

Loop order is tap-outer / chunk-inner: each (image, oc-half) group keeps 7
PSUM banks live (one per 8-row output chunk, N=448 fp32 <= 512 bank limit)
and sweeps the 9 taps over all 7 chunks, so the stationary weight is reused
across 7 consecutive matmuls and LDWEIGHTS hides under the 448-cycle stream.
Measured steady-state matmul issue gap: 192ns (187ns streaming floor); with
per-matmul weight swaps it was 237ns, with fp32r 4-byte weights 253ns.

Head/tail plumbing (all measured bottlenecks in earlier traces):
- weight halves are host-relaid contiguous ([ic, half, tap, 128]) and ride
  the otherwise-idle vector DGE ring, so the first half lands ~1us after
  user code starts instead of queuing behind image bytes.
- image 0 is split into two contiguous row-slabs (rows 0-33 / 32-57) so the
  first group's matmuls gate on a 0.46MB transfer, not the whole 0.86MB
  image; images 1-3 are whole-image DMAs double-buffered on the sync ring.
- PSUM evacuation alternates DVE/ACT per chunk so bank-free keeps pace with
  the next group's matmuls; output DMAs ride the sync ring (the ACT ring
  backed up behind its copies and cost an 8.7us tail when it carried them).
- dummy matmuls on a zeroed scratch tile bridge the initial DMA wait so the
  PE HAM clock gate is at full rate when the real matmuls start.
"""

import ml_dtypes
import numpy as np

import concourse.tile as tile
from concourse import bacc, mybir
from concourse.bass_utils import run_bass_kernel_spmd

N_CORES = 8
B, IC, H, W = 32, 128, 56, 56
OC, KH, KW = 256, 3, 3
BPC = B // N_CORES          # images per core
PH, PW = H + 2, W + 2       # padded 58x58
ROWS_PER_CHUNK = 8
N_CHUNKS = H // ROWS_PER_CHUNK  # 7
OC_HALVES = OC // 128       # 2
NTAPS = KH * KW

_f32 = mybir.dt.float32
_bf16 = mybir.dt.bfloat16
_bf16_np = ml_dtypes.bfloat16

_compiled_nc = None

N_WARMUP = 7  # dummy matmuls covering the initial DMA wait (~2.6us cold)


def _build(warmup=N_WARMUP):
    nc = bacc.Bacc("TRN2", target_bir_lowering=False, debug=False)
    x_d = nc.dram_tensor("x", [BPC, IC, PH, PW], _bf16, kind="ExternalInput")
    w_d = nc.dram_tensor("w", [IC, OC_HALVES * NTAPS * 128], _bf16,
                         kind="ExternalInput")
    o_d = nc.dram_tensor("out", [BPC, OC, H, W], _f32, kind="ExternalOutput")
    w4 = w_d[:].rearrange("p (h k c) -> p h k c", h=OC_HALVES, k=NTAPS, c=128)

    with tile.TileContext(nc) as tc:
        with (
            tc.tile_pool(name="w", bufs=1) as wpool,
            tc.tile_pool(name="x", bufs=1) as xpool,
            tc.tile_pool(name="o", bufs=1) as opool,
            tc.tile_pool(name="ps", bufs=8, space="PSUM") as pspool,
        ):
            if warmup:
                wscr = wpool.tile([128, 128], _bf16, name="wscr", tag="wscr")
                xscr = wpool.tile([128, ROWS_PER_CHUNK * W], _bf16,
                                  name="xscr", tag="xscr")
                nc.gpsimd.memset(wscr[:], 0.0)
                nc.gpsimd.memset(xscr[:], 0.0)
                pwarm = pspool.tile([128, ROWS_PER_CHUNK * W], _f32,
                                    name="pwarm", tag="ps")
                for _ in range(warmup):
                    nc.tensor.matmul(pwarm[:], wscr[:], xscr[:],
                                     start=True, stop=True)

            # sync (HWDGE) ring order = first-needed order: weight half 0,
            # then image-0 in three contiguous row-slabs (each gates only its
            # chunks), then weight half 1, then whole images 1-3
            wh = []
            for half in range(OC_HALVES):
                t = wpool.tile([IC, NTAPS, 128], _bf16, name=f"wh{half}",
                               tag=f"wh{half}")
                wh.append(t)
            nc.sync.dma_start(wh[0][:], w4[:, 0])

            def tap(half, k):
                return wh[half][:, k, :]

            # slabs: rows 0-17 (chunks 0-1), 16-41 (chunks 2-4),
            # 40-57 (chunks 5-6)
            slab_rows = [(0, 18), (16, 26), (40, 18)]
            slabs = []
            for si, (r0, nr) in enumerate(slab_rows):
                s = xpool.tile([IC, nr, PW], _bf16, name=f"x0s{si}",
                               tag=f"x0s{si}")
                nc.sync.dma_start(s[:], x_d[0, :, r0 : r0 + nr, :])
                slabs.append(s)
            nc.sync.dma_start(wh[1][:], w4[:, 1])

            def img0_rhs(ch, kh, kw):
                r = ch * ROWS_PER_CHUNK + kh
                si = 0 if ch < 2 else (1 if ch < 5 else 2)
                r -= slab_rows[si][0]
                return slabs[si][:, r : r + ROWS_PER_CHUNK, kw : kw + W]

            # images 1-3 fully prefetched up front (own buffers, no reuse
            # waits) so no image DMA ever queues behind output DMAs on the
            # sync ring
            rhs_fns = [img0_rhs]
            for img in range(1, BPC):
                xt = xpool.tile([IC, PH, PW], _bf16, name=f"xt{img}",
                                tag=f"xt{img}")
                nc.sync.dma_start(xt[:], x_d[img])

                def rhs_of(ch, kh, kw, _xt=xt):
                    r = ch * ROWS_PER_CHUNK + kh
                    return _xt[:, r : r + ROWS_PER_CHUNK, kw : kw + W]

                rhs_fns.append(rhs_of)

            for img in range(BPC):
                rhs_of = rhs_fns[img]
                for half in range(OC_HALVES):
                    pss = []
                    for ch in range(N_CHUNKS):
                        ps = pspool.tile([128, ROWS_PER_CHUNK, W], _f32,
                                         name="ps", tag="ps")
                        pss.append(ps)
                    for k in range(NTAPS):
                        kh, kw = divmod(k, KW)
                        for ch in range(N_CHUNKS):
                            nc.tensor.matmul(
                                pss[ch][:],
                                tap(half, k),
                                rhs_of(ch, kh, kw),
                                start=(k == 0),
                                stop=(k == NTAPS - 1),
                            )
                    for ch in range(N_CHUNKS):
                        r0 = ch * ROWS_PER_CHUNK
                        if ch % 2 == 0:
                            ot = opool.tile([128, ROWS_PER_CHUNK, W], _f32,
                                            name="otv", tag="otv", bufs=3)
                            nc.vector.tensor_copy(ot[:], pss[ch][:])
                            ring = nc.sync
                        else:
                            ot = opool.tile([128, ROWS_PER_CHUNK, W], _f32,
                                            name="ots", tag="ots", bufs=3)
                            nc.scalar.copy(ot[:], pss[ch][:])
                            ring = nc.scalar
                        ring.dma_start(
                            o_d[img, half * 128 : half * 128 + 128,
                                r0 : r0 + ROWS_PER_CHUNK, :],
                            ot[:],
                        )
    nc.compile()
    return nc


def _get_nc():
    global _compiled_nc
    if _compiled_nc is None:
        _compiled_nc = _build()
    return _compiled_nc


def _prep_inputs(x, weight):
    x = np.asarray(x, dtype=np.float32)
    weight = np.asarray(weight, dtype=np.float32)
    xp = np.zeros((B, IC, PH, PW), dtype=_bf16_np)
    xp[:, :, 1 : H + 1, 1 : W + 1] = x
    # [oc, ic, kh, kw] -> [ic, oc-half, kh*kw, 128] -> [ic, flat]
    wt = weight.transpose(1, 0, 2, 3).reshape(IC, OC_HALVES, 128, NTAPS)
    wt = np.ascontiguousarray(wt.transpose(0, 1, 3, 2).astype(_bf16_np))
    wt = wt.reshape(IC, OC_HALVES * NTAPS * 128)
    in_maps = [
        {"x": np.ascontiguousarray(xp[c * BPC : (c + 1) * BPC]), "w": wt}
        for c in range(N_CORES)
    ]
    return in_maps


def _run(x, weight, trace=False):
    nc = _get_nc()
    in_maps = _prep_inputs(x, weight)
    res = run_bass_kernel_spmd(nc, in_maps, list(range(N_CORES)), trace=trace)
    out = np.concatenate([res.results[c]["out"] for c in range(N_CORES)], axis=0)
    return out, res


def kernel(x, weight):
    out, _ = _run(x, weight)
    return out


# revision 19
# speedup vs baseline: 1.2621x; 1.0059x over previous
"""Trainium2 Bass kernel for nn_Conv2d_22222160789797.

Conv2d: x [32,128,56,56] f32, weight [256,128,3,3] (OIHW), stride 1, pad 1
-> out [32,256,56,56] f32.

Strategy: data-parallel over batch across 8 cores (4 images/core). Per core,
the conv is 9 accumulating matmuls per output tile: contract over in-channels
(partition dim K=128) with the weight slice for each (kh,kw) tap as the
stationary operand and a shifted window of the zero-padded input as the moving
operand. Inputs are cast to bf16 on the host (~2e-3 max rel err, limit 2e-2).

Loop order is tap-outer / chunk-inner: each (image, oc-half) group keeps 7
PSUM banks live (one per 8-row output chunk, N=448 fp32 <= 512 bank limit)
and sweeps the 9 taps over all 7 chunks, so the stationary weight is reused
across 7 consecutive matmuls and LDWEIGHTS hides under the 448-cycle stream.
Measured steady-state matmul issue gap: 192ns (187ns streaming floor); with
per-matmul weight swaps it was 237ns, with fp32r 4-byte weights 253ns.

Head/tail plumbing (all measured bottlenecks in earlier traces):
- sync (HWDGE) ring carries input loads in first-needed order: weight half 0
  (host-relaid so each half is contiguous, 2304B/partition), image-0 as
  three contiguous row-slabs (each gates only its 2-3 chunks), weight half
  1, then images 1-3 whole (6728B/partition) into dedicated buffers so no
  image DMA ever queues behind output DMAs. SWDGE (gpsimd) and the ACT ring
  measured 2-5us slower to first byte for the critical head transfers.
- PSUM evacuation alternates DVE/ACT per chunk so bank-free keeps pace with
  the next group's matmuls; output DMAs split across the sync/ACT rings
  (even/odd chunks) to halve the end-of-kernel issue serialization. Outputs
  are written bf16 (host upcasts) to halve the output-drain time.
- dummy matmuls on a zeroed scratch tile bridge the initial DMA wait so the
  PE HAM clock gate is at full rate when the real matmuls start (a >3.4us
  PE-idle gap re-throttles the clock to 1.2GHz for ~4us).
"""

import ml_dtypes
import numpy as np

import concourse.tile as tile
from concourse import bacc, mybir
from concourse.bass_utils import run_bass_kernel_spmd

N_CORES = 8
B, IC, H, W = 32, 128, 56, 56
OC, KH, KW = 256, 3, 3
BPC = B // N_CORES          # images per core
PH, PW = H + 2, W + 2       # padded 58x58
ROWS_PER_CHUNK = 8
N_CHUNKS = H // ROWS_PER_CHUNK  # 7
OC_HALVES = OC // 128       # 2
NTAPS = KH * KW

_f32 = mybir.dt.float32
_bf16 = mybir.dt.bfloat16
_bf16_np = ml_dtypes.bfloat16

_compiled_nc = None

N_WARMUP = 9  # dummy matmuls covering the initial DMA wait (~3.3us cold)


def _build(warmup=N_WARMUP):
    nc = bacc.Bacc("TRN2", target_bir_lowering=False, debug=False)
    x_d = nc.dram_tensor("x", [BPC, IC, PH, PW], _bf16, kind="ExternalInput")
    w_d = nc.dram_tensor("w", [IC, OC_HALVES * NTAPS * 128], _bf16,
                         kind="ExternalInput")
    o_d = nc.dram_tensor("out", [BPC, OC, H, W], _bf16, kind="ExternalOutput")
    w4 = w_d[:].rearrange("p (h k c) -> p h k c", h=OC_HALVES, k=NTAPS, c=128)

    with tile.TileContext(nc) as tc:
        with (
            tc.tile_pool(name="w", bufs=1) as wpool,
            tc.tile_pool(name="x", bufs=1) as xpool,
            tc.tile_pool(name="o", bufs=1) as opool,
            tc.tile_pool(name="ps", bufs=8, space="PSUM") as pspool,
        ):
            if warmup:
                wscr = wpool.tile([128, 128], _bf16, name="wscr", tag="wscr")
                xscr = wpool.tile([128, ROWS_PER_CHUNK * W], _bf16,
                                  name="xscr", tag="xscr")
                nc.gpsimd.memset(wscr[:], 0.0)
                nc.gpsimd.memset(xscr[:], 0.0)
                pwarm = pspool.tile([128, ROWS_PER_CHUNK * W], _f32,
                                    name="pwarm", tag="ps")
                for _ in range(warmup):
                    nc.tensor.matmul(pwarm[:], wscr[:], xscr[:],
                                     start=True, stop=True)

            # sync (HWDGE) ring order = first-needed order: weight half 0,
            # then image-0 in three contiguous row-slabs (each gates only its
            # chunks), then weight half 1, then whole images 1-3
            wh = []
            for half in range(OC_HALVES):
                t = wpool.tile([IC, NTAPS, 128], _bf16, name=f"wh{half}",
                               tag=f"wh{half}")
                wh.append(t)
            nc.sync.dma_start(wh[0][:], w4[:, 0])

            def tap(half, k):
                return wh[half][:, k, :]

            # slabs: rows 0-17 (chunks 0-1), 16-41 (chunks 2-4),
            # 40-57 (chunks 5-6)
            slab_rows = [(0, 18), (16, 26), (40, 18)]
            slabs = []
            for si, (r0, nr) in enumerate(slab_rows):
                s = xpool.tile([IC, nr, PW], _bf16, name=f"x0s{si}",
                               tag=f"x0s{si}")
                nc.sync.dma_start(s[:], x_d[0, :, r0 : r0 + nr, :])
                slabs.append(s)
            nc.sync.dma_start(wh[1][:], w4[:, 1])

            def img0_rhs(ch, kh, kw):
                r = ch * ROWS_PER_CHUNK + kh
                si = 0 if ch < 2 else (1 if ch < 5 else 2)
                r -= slab_rows[si][0]
                return slabs[si][:, r : r + ROWS_PER_CHUNK, kw : kw + W]

            # images 1-3 fully prefetched up front (own buffers, no reuse
            # waits) so no image DMA ever queues behind output DMAs on the
            # sync ring
            rhs_fns = [img0_rhs]
            for img in range(1, BPC):
                xt = xpool.tile([IC, PH, PW], _bf16, name=f"xt{img}",
                                tag=f"xt{img}")
                nc.sync.dma_start(xt[:], x_d[img])

                def rhs_of(ch, kh, kw, _xt=xt):
                    r = ch * ROWS_PER_CHUNK + kh
                    return _xt[:, r : r + ROWS_PER_CHUNK, kw : kw + W]

                rhs_fns.append(rhs_of)

            for img in range(BPC):
                rhs_of = rhs_fns[img]
                for half in range(OC_HALVES):
                    pss = []
                    for ch in range(N_CHUNKS):
                        ps = pspool.tile([128, ROWS_PER_CHUNK, W], _f32,
                                         name="ps", tag="ps")
                        pss.append(ps)
                    for k in range(NTAPS):
                        kh, kw = divmod(k, KW)
                        for ch in range(N_CHUNKS):
                            nc.tensor.matmul(
                                pss[ch][:],
                                tap(half, k),
                                rhs_of(ch, kh, kw),
                                start=(k == 0),
                                stop=(k == NTAPS - 1),
                            )
                    for ch in range(N_CHUNKS):
                        r0 = ch * ROWS_PER_CHUNK
                        if ch % 2 == 0:
                            ot = opool.tile([128, ROWS_PER_CHUNK, W], _bf16,
                                            name="otv", tag="otv", bufs=4)
                            nc.vector.tensor_copy(ot[:], pss[ch][:])
                            ring = nc.sync
                        else:
                            ot = opool.tile([128, ROWS_PER_CHUNK, W], _bf16,
                                            name="ots", tag="ots", bufs=4)
                            nc.scalar.copy(ot[:], pss[ch][:])
                            ring = nc.scalar
                        ring.dma_start(
                            o_d[img, half * 128 : half * 128 + 128,
                                r0 : r0 + ROWS_PER_CHUNK, :],
                            ot[:],
                        )
    nc.compile()
    return nc


def _get_nc():
    global _compiled_nc
    if _compiled_nc is None:
        _compiled_nc = _build()
    return _compiled_nc


def _prep_inputs(x, weight):
    x = np.asarray(x, dtype=np.float32)
    weight = np.asarray(weight, dtype=np.float32)
    xp = np.zeros((B, IC, PH, PW), dtype=_bf16_np)
    xp[:, :, 1 : H + 1, 1 : W + 1] = x
    # [oc, ic, kh, kw] -> [ic, oc-half, kh*kw, 128] -> [ic, flat]
    wt = weight.transpose(1, 0, 2, 3).reshape(IC, OC_HALVES, 128, NTAPS)
    wt = np.ascontiguousarray(wt.transpose(0, 1, 3, 2).astype(_bf16_np))
    wt = wt.reshape(IC, OC_HALVES * NTAPS * 128)
    in_maps = [
        {"x": np.ascontiguousarray(xp[c * BPC : (c + 1) * BPC]), "w": wt}
        for c in range(N_CORES)
    ]
    return in_maps


def _run(x, weight, trace=False):
    nc = _get_nc()
    in_maps = _prep_inputs(x, weight)
    res = run_bass_kernel_spmd(nc, in_maps, list(range(N_CORES)), trace=trace)
    out = np.concatenate(
        [np.asarray(res.results[c]["out"]) for c in range(N_CORES)], axis=0
    ).astype(np.float32)
    return out, res


def kernel(x, weight):
    out, _ = _run(x, weight)
    return out
